# revision 60
# baseline (speedup 1.0000x reference)
"""Enformer dot-product self-attention with central-mask relative position
bias, on 8 Trainium2 NeuronCores (one head per core, SPMD).

Math per head h (S=2048, D=64, N=64):
    basis[i,j,:] = f(d=i-j)  — indicator features, zero for |d| > 1024
    logits = (q @ k^T + (q @ w) @ basis^T + u @ k^T + (v @ w) @ basis^T) / 8
    out    = softmax(logits) @ value

Formulation (transposed logits, block schedule, all-bf16 matmuls):
  - qT_aug [65,S] bf16: rows 0..63 = (q/8)^T, row 64 = ones.  Shared by the
    band matmuls (lhsT) and the qk matmuls (rhs).
  - k_aug [65,S] bf16: rows 0..63 = k^T (loaded), row 64 = (u/8)@k^T
    (computed on device).  logits^T tile [128 j, i] =
    matmul(lhsT=k_aug[:, jtile], rhs=qT_aug).
  - Band term: T[i,c'] = qT_aug[:,i] . w2r[:,c'], c' = j-i+1024, w2r[65,2176]
    host-built.  A-block: T rows -> DRAM G (pitch Q), in-band cols [clo,chi);
    the [2049, Q) zero pad is written once up front (the shear's negative-c'
    and far-edge excursions land on pad columns).  C-block: per j-tile ONE
    DMA-xbar read (transpose=True) of the flat diagonal pattern (row step
    Q-1) delivers the sheared band already transposed: bt[j,i] = band[i,j].
  - softmax: DVE adds bt onto the qk PSUM half, ScalarE exp (no max
    subtraction, no accum_out), P^T bf16 stored [128 j, i].
  - attn@v: out^T[d, i-chunk] accumulated in PSUM over j-tiles with
    lhsT = V_aug[jtile] [128, 65] (col 64 = ones -> row 64 = denominators).
  - Final: PE transpose of out^T chunks, multiply by 1/denominator, store.
  - Pools sized to avoid head-of-line cycles on the sync DMA queue
    (btsb=9 covers the full 8-tile xbar lead; gsb=6 covers G-write backlog).
"""

import numpy as np
import ml_dtypes

import concourse.bass as bass
import concourse.bacc as bacc
import concourse.mybir as mybir
import concourse.tile as tile
from concourse.bass_utils import run_bass_kernel_spmd
from concourse.masks import make_identity

S = 2048
D = 64
NB = 64          # pos-emb dim (basis features)
H = 8
HALF = NB // 2   # 32
BAND = 1024      # max |d| with nonzero features
Q = S + 128      # G row pitch (2049 band cols + 127 zero pad)
F32 = mybir.dt.float32
BF16 = mybir.dt.bfloat16

_NC_CACHE = {}


def _basis_feature_matrix():
    """Rr[c', n] for c' in [0, Q): features of distance d = 1024 - c'.
    Matches reference._relative_basis numerics (float32)."""
    pow_rate = np.float32(np.exp(np.log((S + 1) / 2) / HALF))
    widths = np.power(pow_rate, np.arange(1, HALF + 1, dtype=np.float32),
                      dtype=np.float32)  # [32]
    d = (np.float32(BAND) - np.arange(Q, dtype=np.float32))[:, None]  # [Q,1]
    unsigned = (np.abs(d) <= widths[None, :]).astype(np.float32)      # [Q,32]
    signed = np.sign(d) * unsigned
    return np.concatenate([unsigned, signed], axis=-1)  # [Q, 64]


def _clo_chi(t):
    i0 = t * 128
    jlo = max(0, i0 - BAND)
    jhi = min(S, i0 + 128 + BAND)
    clo = max(0, (jlo - i0 + BAND) - 127)
    chi = min(2049, (jhi - 1) - i0 + BAND + 1)
    return clo, chi


def _build_nc():
    if "nc" in _NC_CACHE:
        return _NC_CACHE["nc"]

    nc = bacc.Bacc("TRN2", target_bir_lowering=False, debug=False,
                   num_devices=H)
    d_kT = nc.dram_tensor("kTb", [D, S], BF16, kind="ExternalInput")
    d_u8 = nc.dram_tensor("u8b", [D, 1], BF16, kind="ExternalInput")
    d_v = nc.dram_tensor("vb", [S, D], BF16, kind="ExternalInput")
    d_w2r = nc.dram_tensor("w2r", [65, Q], BF16, kind="ExternalInput")
    d_qTb = nc.dram_tensor("qT_bf", [65, S], BF16, kind="ExternalInput")
    d_out = nc.dram_tensor("out", [S, D], F32, kind="ExternalOutput")

    NT = S // 128  # 16 i/j tiles
    d_G = nc.dram_tensor("gband", [S * Q], BF16, kind="Internal")

    with tile.TileContext(nc) as tc:
        with tc.tile_pool(name="pers", bufs=1) as pers:
            # sync queue: exactly what A(0) needs first, then the rest.
            sb_qTb = pers.tile([65, S], BF16)
            nc.sync.dma_start(out=sb_qTb[:, 0:128], in_=d_qTb[:, 0:128])
            sb_w2r = pers.tile([65, Q], BF16)
            nc.sync.dma_start(out=sb_w2r[:, 897:1473], in_=d_w2r[:, 897:1473])
            nc.sync.dma_start(out=sb_w2r[:, 1473:2049],
                              in_=d_w2r[:, 1473:2049])
            nc.sync.dma_start(out=sb_qTb[:, 128:640], in_=d_qTb[:, 128:640])
            nc.sync.dma_start(out=sb_w2r[:, 0:897], in_=d_w2r[:, 0:897])
            nc.sync.dma_start(out=sb_qTb[:, 640:1664], in_=d_qTb[:, 640:1664])
            nc.sync.dma_start(out=sb_qTb[:, 1664:S], in_=d_qTb[:, 1664:S])
            # One-time zero fill of the G pad columns [2049, Q), all rows —
            # the shear reads them both directly (far-band edge) and via
            # negative-c' excursions onto the previous row's pad.
            sb_zpad = pers.tile([128, 16, 127], BF16)
            nc.gpsimd.memset(sb_zpad[:], 0.0)
            padw = bass.AP(tensor=d_G, offset=2049,
                           ap=[[Q, 2048], [1, 127]])
            nc.scalar.dma_start(out=padw, in_=sb_zpad[:])

            # scalar queue: k/u inputs (needed by the u@k row setup).
            sb_kaug = pers.tile([65, S], BF16)
            for c in range(2):
                nc.scalar.dma_start(out=sb_kaug[0:D, c * 1024:(c + 1) * 1024],
                                    in_=d_kT[:, c * 1024:(c + 1) * 1024])
            sb_u8 = pers.tile([D, 1], BF16)
            nc.scalar.dma_start(out=sb_u8[:], in_=d_u8[:])

            # gpsimd queue: V (augmented with a ones column), identity.
            sb_vaug = pers.tile([128, NT, 65], BF16)
            nc.gpsimd.memset(sb_vaug[:], 1.0)
            for t in range(NT):
                nc.gpsimd.dma_start(out=sb_vaug[:, t, 0:D],
                                    in_=d_v[t * 128:(t + 1) * 128, :])
            sb_id = pers.tile([128, 128], F32)
            make_identity(nc, sb_id[:])

            sb_PT = pers.tile([128, NT, S], BF16)     # exp(logits^T), j-tiled
            sb_oTD = pers.tile([65, S], F32)          # out^T rows + den row 64

            with tc.tile_pool(name="gsb", bufs=8) as gsb, \
                 tc.tile_pool(name="btsb", bufs=11) as btsb, \
                 tc.tile_pool(name="fsb", bufs=8) as fsb:

                # A block owns ALL 8 PSUM banks; pool closes before the C
                # block opens its own (8-deep rotation + light evac queues
                # -> no matmul waits -> PE p-state ramps to full clock).
                psA_ctx = tc.tile_pool(name="psA", bufs=8, space="PSUM")
                psA = psA_ctx.__enter__()

                def phase_A(t):
                    i0 = t * 128
                    clo, chi = _clo_chi(t)
                    gt = gsb.tile([128, 2049], BF16)
                    cuts = list(range(clo, chi, 512)) + [chi]
                    for ci in range(len(cuts) - 1):
                        lo, hi = cuts[ci], cuts[ci + 1]
                        pg = psA.tile([128, 512], F32, tag="x")
                        nc.tensor.matmul(
                            pg[:, 0:hi - lo],
                            lhsT=sb_qTb[:, i0:i0 + 128],
                            rhs=sb_w2r[:, lo:hi],
                            start=True, stop=True)
                        # one whole-chunk evac, engines alternating: fewer
                        # ops per engine keeps the scalar queue off the
                        # critical path (sem overhead is per-op)
                        if ci % 2 == 0 and t >= 1:
                            nc.scalar.copy(out=gt[:, lo:hi],
                                           in_=pg[:, 0:hi - lo])
                        else:
                            nc.vector.tensor_copy(gt[:, lo:hi],
                                                  pg[:, 0:hi - lo])
                    wr = bass.AP(tensor=d_G, offset=i0 * Q + clo,
                                 ap=[[Q, 128], [1, chi - clo]])
                    nc.sync.dma_start(out=wr, in_=gt[:, clo:chi])

                def phase_X(tj):
                    # xbar-transposed shear read: bt[jj, i] = band[i, j0+jj]
                    j0 = tj * 128
                    ilo = max(0, j0 - BAND)
                    ihi = min(S, j0 + 128 + BAND)
                    bt = btsb.tile([128, S], BF16)
                    rd = bass.AP(tensor=d_G,
                                 offset=ilo * Q + (j0 - ilo + BAND),
                                 ap=[[Q - 1, ihi - ilo], [1, 128]])
                    nc.sync.dma_start(out=bt[:, ilo:ihi], in_=rd,
                                      transpose=True)
                    return bt, ilo, ihi

                def phase_C(tj, bt, ilo, ihi):
                    j0 = tj * 128
                    for h in range(4):
                        pq = ps2.tile([128, 512], F32, tag="x")
                        nc.tensor.matmul(
                            pq[:],
                            lhsT=sb_kaug[:, j0:j0 + 128],
                            rhs=sb_qTb[:, h * 512:(h + 1) * 512],
                            start=True, stop=True)
                        alo = max(ilo, h * 512)
                        ahi = min(ihi, (h + 1) * 512)
                        if alo < ahi:
                            nc.vector.tensor_add(
                                pq[:, alo - h * 512:ahi - h * 512],
                                pq[:, alo - h * 512:ahi - h * 512],
                                bt[:, alo:ahi])
                        nc.scalar.activation(
                            out=sb_PT[:, tj, h * 512:(h + 1) * 512],
                            in_=pq[:],
                            func=mybir.ActivationFunctionType.Exp)

                def phase_AV(tj, po):
                    for ic in range(4):
                        nc.tensor.matmul(
                            po[ic][:],
                            lhsT=sb_vaug[:, tj, :],
                            rhs=sb_PT[:, tj, ic * 512:(ic + 1) * 512],
                            start=(tj == 0), stop=(tj == NT - 1))

                # ---- A block (lead: 12 of 16 tiles) ----
                phase_A(0)
                for c in range(4):
                    pk = psA.tile([128, 512], F32, tag="x")
                    nc.tensor.matmul(
                        pk[0:1, 0:512],
                        lhsT=sb_u8[:],
                        rhs=sb_kaug[0:D, c * 512:(c + 1) * 512],
                        start=True, stop=True)
                    nc.scalar.copy(
                        out=sb_kaug[64:65, c * 512:(c + 1) * 512],
                        in_=pk[0:1, 0:512])
                bts = {}
                for t in range(1, NT):
                    if t >= 9:
                        # issue the xbar ahead of this tile's G write so the
                        # C block's band supply is never queued behind it
                        bts[t - 9] = phase_X(t - 9)
                    phase_A(t)
                bts[7] = phase_X(7)
                psA_ctx.__exit__(None, None, None)

                # ---- C block ----
                ps2_ctx = tc.tile_pool(name="ps2", bufs=4, space="PSUM")
                ps2 = ps2_ctx.__enter__()
                pso_ctx = tc.tile_pool(name="ps_o", bufs=1, space="PSUM")
                pso = pso_ctx.__enter__()
                po = [pso.tile([65, 512], F32, tag=f"po{ic}",
                               name=f"po{ic}")
                      for ic in range(4)]
                # attn@v runs one j-tile behind the logits pipeline so its
                # exp inputs are always ready when the PE reaches it.
                for tj in range(NT):
                    if tj + 8 < NT:
                        bts[tj + 8] = phase_X(tj + 8)
                    phase_C(tj, *bts.pop(tj))
                    if tj >= 2:
                        phase_AV(tj - 2, po)
                phase_AV(NT - 2, po)
                phase_AV(NT - 1, po)

                # ---- output fixup ----
                # DVE: po evac + reciprocals; ScalarE: 1/den multiplies
                # (reads pf straight from PSUM); sync: output writes.
                for ic in range(4):
                    nc.vector.tensor_copy(
                        sb_oTD[:, ic * 512:(ic + 1) * 512], po[ic][:])
                for t in range(NT):
                    pf = pso.tile([128, 65], F32, tag=f"po{t % 4}")
                    nc.tensor.transpose(pf[:, 0:65],
                                        sb_oTD[:, t * 128:(t + 1) * 128],
                                        sb_id[0:65, 0:65])
                    rc = fsb.tile([128, 1], F32, tag="rc")
                    nc.vector.reciprocal(rc[:], pf[:, 64:65])
                    ot = fsb.tile([128, D], F32, tag="ot")
                    nc.scalar.mul(ot[:], pf[:, 0:D], rc[:])
                    nc.sync.dma_start(out=d_out[t * 128:(t + 1) * 128, :],
                                      in_=ot[:])
                pso_ctx.__exit__(None, None, None)
                ps2_ctx.__exit__(None, None, None)

    nc.finalize()
    _NC_CACHE["nc"] = nc
    return nc


def _host_prep(query, key, value, u, v, w):
    """Build the 8 per-core input maps from the full inputs."""
    q = np.asarray(query, np.float32)[0]   # [S,H,D]
    k = np.asarray(key, np.float32)[0]
    val = np.asarray(value, np.float32)[0]
    u = np.asarray(u, np.float32)
    v = np.asarray(v, np.float32)
    w = np.asarray(w, np.float32)
    Rr = _basis_feature_matrix()           # [Q, 64]

    ones = np.ones((1, S), np.float32)
    in_maps = []
    for h in range(H):
        qT8 = np.ascontiguousarray(q[:, h, :].T) / np.float32(8.0)  # [64,S]
        qT_aug = np.concatenate([qT8, ones], axis=0)                # [65,S]
        kT = np.ascontiguousarray(k[:, h, :].T)                     # [64,S]
        u8 = (u[h] / np.float32(8.0)).reshape(D, 1)
        vb = val[:, h, :].astype(ml_dtypes.bfloat16)                # [S,64]
        w2r_qr = w[h] @ Rr.T                                        # [64,Q]
        vw8 = (v[h] @ w[h]) / np.float32(8.0)                       # [64]
        w2r_vr = (vw8 @ Rr.T).reshape(1, Q)                         # [1,Q]
        w2r = np.concatenate([w2r_qr, w2r_vr],
                             axis=0).astype(ml_dtypes.bfloat16)
        in_maps.append({
            "qT_bf": np.ascontiguousarray(qT_aug).astype(ml_dtypes.bfloat16),
            "kTb": kT.astype(ml_dtypes.bfloat16),
            "u8b": np.ascontiguousarray(u8).astype(ml_dtypes.bfloat16),
            "vb": np.ascontiguousarray(vb),
            "w2r": np.ascontiguousarray(w2r),
        })
    return in_maps


def kernel(query, key, value, u, v, w, _trace=False):
    nc = _build_nc()
    in_maps = _host_prep(query, key, value, u, v, w)
    res = run_bass_kernel_spmd(nc, in_maps, core_ids=list(range(H)),
                               trace=_trace)
    outs = np.stack([res.results[h]["out"] for h in range(H)])  # [H,S,D]
    full = np.transpose(outs, (1, 0, 2))[None]                  # [1,S,H,D]
    out = np.ascontiguousarray(full.astype(np.float32))
    if _trace:
        return out, res
    return out


if __name__ == "__main__":
    rng = np.random.default_rng(0)
    ins = {
        "query": rng.standard_normal((1, S, H, D), np.float32),
        "key": rng.standard_normal((1, S, H, D), np.float32),
        "value": rng.standard_normal((1, S, H, D), np.float32),
        "u": rng.standard_normal((H, D), np.float32),
        "v": rng.standard_normal((H, D), np.float32),
        "w": rng.standard_normal((H, D, NB), np.float32),
    }
    out = kernel(**ins)
    print("out shape:", out.shape, "finite:", np.isfinite(out).all())


# revision 61
# speedup vs baseline: 1.0140x; 1.0140x over previous
"""Enformer dot-product self-attention with central-mask relative position
bias, on 8 Trainium2 NeuronCores (one head per core, SPMD).

Math per head h (S=2048, D=64, N=64):
    basis[i,j,:] = f(d=i-j)  — indicator features, zero for |d| > 1024
    logits = (q @ k^T + (q @ w) @ basis^T + u @ k^T + (v @ w) @ basis^T) / 8
    out    = softmax(logits) @ value

Formulation (transposed logits, block schedule, all-bf16 matmuls):
  - qT_aug [65,S] bf16: rows 0..63 = (q/8)^T, row 64 = ones.  Shared by the
    band matmuls (lhsT) and the qk matmuls (rhs).
  - k_aug [65,S] bf16: rows 0..63 = k^T (loaded), row 64 = (u/8)@k^T
    (computed on device).  logits^T tile [128 j, i] =
    matmul(lhsT=k_aug[:, jtile], rhs=qT_aug).
  - Band term: T[i,c'] = qT_aug[:,i] . w2r[:,c'], c' = j-i+1024, w2r[65,2176]
    host-built.  A-block: T rows -> DRAM G (pitch Q), in-band cols [clo,chi);
    the [2049, Q) zero pad is written once up front (the shear's negative-c'
    and far-edge excursions land on pad columns).  C-block: per j-tile ONE
    DMA-xbar read (transpose=True) of the flat diagonal pattern (row step
    Q-1) delivers the sheared band already transposed: bt[j,i] = band[i,j].
  - softmax: DVE adds bt onto the qk PSUM half, ScalarE exp (no max
    subtraction, no accum_out), P^T bf16 stored [128 j, i].
  - attn@v: out^T[d, i-chunk] accumulated in PSUM over j-tiles with
    lhsT = V_aug[jtile] [128, 65] (col 64 = ones -> row 64 = denominators).
  - Final: PE transpose of out^T chunks, multiply by 1/denominator, store.
  - Pools sized to avoid head-of-line cycles on the sync DMA queue
    (btsb=9 covers the full 8-tile xbar lead; gsb=6 covers G-write backlog).
"""

import numpy as np
import ml_dtypes

import concourse.bass as bass
import concourse.bacc as bacc
import concourse.mybir as mybir
import concourse.tile as tile
from concourse.bass_utils import run_bass_kernel_spmd
from concourse.masks import make_identity

S = 2048
D = 64
NB = 64          # pos-emb dim (basis features)
H = 8
HALF = NB // 2   # 32
BAND = 1024      # max |d| with nonzero features
Q = S + 128      # G row pitch (2049 band cols + 127 zero pad)
F32 = mybir.dt.float32
BF16 = mybir.dt.bfloat16

_NC_CACHE = {}


def _basis_feature_matrix():
    """Rr[c', n] for c' in [0, Q): features of distance d = 1024 - c'.
    Matches reference._relative_basis numerics (float32)."""
    pow_rate = np.float32(np.exp(np.log((S + 1) / 2) / HALF))
    widths = np.power(pow_rate, np.arange(1, HALF + 1, dtype=np.float32),
                      dtype=np.float32)  # [32]
    d = (np.float32(BAND) - np.arange(Q, dtype=np.float32))[:, None]  # [Q,1]
    unsigned = (np.abs(d) <= widths[None, :]).astype(np.float32)      # [Q,32]
    signed = np.sign(d) * unsigned
    return np.concatenate([unsigned, signed], axis=-1)  # [Q, 64]


def _clo_chi(t):
    i0 = t * 128
    jlo = max(0, i0 - BAND)
    jhi = min(S, i0 + 128 + BAND)
    clo = max(0, (jlo - i0 + BAND) - 127)
    chi = min(2049, (jhi - 1) - i0 + BAND + 1)
    return clo, chi


def _build_nc():
    if "nc" in _NC_CACHE:
        return _NC_CACHE["nc"]

    nc = bacc.Bacc("TRN2", target_bir_lowering=False, debug=False,
                   num_devices=H)
    d_kT = nc.dram_tensor("kTb", [D, S], BF16, kind="ExternalInput")
    d_u8 = nc.dram_tensor("u8b", [D, 1], BF16, kind="ExternalInput")
    d_v = nc.dram_tensor("vb", [S, D], BF16, kind="ExternalInput")
    d_w2r = nc.dram_tensor("w2r", [65, Q], BF16, kind="ExternalInput")
    d_qTb = nc.dram_tensor("qT_bf", [65, S], BF16, kind="ExternalInput")
    d_out = nc.dram_tensor("out", [S, D], F32, kind="ExternalOutput")

    NT = S // 128  # 16 i/j tiles
    d_G = nc.dram_tensor("gband", [S * Q], BF16, kind="Internal")

    with tile.TileContext(nc) as tc:
        with tc.tile_pool(name="pers", bufs=1) as pers:
            # sync queue: exactly what A(0) needs first, then the rest.
            sb_qTb = pers.tile([65, S], BF16)
            nc.sync.dma_start(out=sb_qTb[:, 0:128], in_=d_qTb[:, 0:128])
            sb_w2r = pers.tile([65, Q], BF16)
            nc.sync.dma_start(out=sb_w2r[:, 897:1473], in_=d_w2r[:, 897:1473])
            nc.sync.dma_start(out=sb_w2r[:, 1473:2049],
                              in_=d_w2r[:, 1473:2049])
            nc.sync.dma_start(out=sb_qTb[:, 128:640], in_=d_qTb[:, 128:640])
            nc.sync.dma_start(out=sb_w2r[:, 0:897], in_=d_w2r[:, 0:897])
            nc.sync.dma_start(out=sb_qTb[:, 640:1664], in_=d_qTb[:, 640:1664])
            nc.sync.dma_start(out=sb_qTb[:, 1664:S], in_=d_qTb[:, 1664:S])
            # One-time zero fill of the G pad columns [2049, Q), all rows —
            # the shear reads them both directly (far-band edge) and via
            # negative-c' excursions onto the previous row's pad.
            sb_zpad = pers.tile([128, 16, 127], BF16)
            nc.gpsimd.memset(sb_zpad[:], 0.0)
            padw = bass.AP(tensor=d_G, offset=2049,
                           ap=[[Q, 2048], [1, 127]])
            nc.scalar.dma_start(out=padw, in_=sb_zpad[:])

            # scalar queue: k/u inputs (needed by the u@k row setup).
            sb_kaug = pers.tile([65, S], BF16)
            for c in range(2):
                nc.scalar.dma_start(out=sb_kaug[0:D, c * 1024:(c + 1) * 1024],
                                    in_=d_kT[:, c * 1024:(c + 1) * 1024])
            sb_u8 = pers.tile([D, 1], BF16)
            nc.scalar.dma_start(out=sb_u8[:], in_=d_u8[:])

            # gpsimd queue: V (augmented with a ones column), identity.
            sb_vaug = pers.tile([128, NT, 65], BF16)
            nc.gpsimd.memset(sb_vaug[:], 1.0)
            for t in range(NT):
                nc.gpsimd.dma_start(out=sb_vaug[:, t, 0:D],
                                    in_=d_v[t * 128:(t + 1) * 128, :])
            sb_id = pers.tile([128, 128], F32)
            make_identity(nc, sb_id[:])

            sb_PT = pers.tile([128, NT, S], BF16)     # exp(logits^T), j-tiled
            sb_oTD = pers.tile([65, S], F32)          # out^T rows + den row 64

            with tc.tile_pool(name="gsb", bufs=8) as gsb, \
                 tc.tile_pool(name="btsb", bufs=11) as btsb, \
                 tc.tile_pool(name="fsb", bufs=8) as fsb:

                # A block owns ALL 8 PSUM banks; pool closes before the C
                # block opens its own (8-deep rotation + light evac queues
                # -> no matmul waits -> PE p-state ramps to full clock).
                psA_ctx = tc.tile_pool(name="psA", bufs=8, space="PSUM")
                psA = psA_ctx.__enter__()

                def phase_A(t):
                    i0 = t * 128
                    clo, chi = _clo_chi(t)
                    gt = gsb.tile([128, 2049], BF16)
                    cuts = list(range(clo, chi, 512)) + [chi]
                    for ci in range(len(cuts) - 1):
                        lo, hi = cuts[ci], cuts[ci + 1]
                        pg = psA.tile([128, 512], F32, tag="x")
                        nc.tensor.matmul(
                            pg[:, 0:hi - lo],
                            lhsT=sb_qTb[:, i0:i0 + 128],
                            rhs=sb_w2r[:, lo:hi],
                            start=True, stop=True)
                        # one whole-chunk evac, engines alternating: fewer
                        # ops per engine keeps the scalar queue off the
                        # critical path (sem overhead is per-op)
                        if ci % 2 == 0 and t >= 1:
                            nc.scalar.copy(out=gt[:, lo:hi],
                                           in_=pg[:, 0:hi - lo])
                        else:
                            nc.vector.tensor_copy(gt[:, lo:hi],
                                                  pg[:, 0:hi - lo])
                    wr = bass.AP(tensor=d_G, offset=i0 * Q + clo,
                                 ap=[[Q, 128], [1, chi - clo]])
                    nc.sync.dma_start(out=wr, in_=gt[:, clo:chi])

                def phase_X(tj):
                    # xbar-transposed shear read: bt[jj, i] = band[i, j0+jj]
                    j0 = tj * 128
                    ilo = max(0, j0 - BAND)
                    ihi = min(S, j0 + 128 + BAND)
                    bt = btsb.tile([128, S], BF16)
                    rd = bass.AP(tensor=d_G,
                                 offset=ilo * Q + (j0 - ilo + BAND),
                                 ap=[[Q - 1, ihi - ilo], [1, 128]])
                    nc.sync.dma_start(out=bt[:, ilo:ihi], in_=rd,
                                      transpose=True)
                    return bt, ilo, ihi

                def phase_C(tj, bt, ilo, ihi):
                    j0 = tj * 128
                    for h in range(4):
                        pq = ps2.tile([128, 512], F32, tag="x")
                        nc.tensor.matmul(
                            pq[:],
                            lhsT=sb_kaug[:, j0:j0 + 128],
                            rhs=sb_qTb[:, h * 512:(h + 1) * 512],
                            start=True, stop=True)
                        alo = max(ilo, h * 512)
                        ahi = min(ihi, (h + 1) * 512)
                        if alo < ahi:
                            nc.vector.tensor_add(
                                pq[:, alo - h * 512:ahi - h * 512],
                                pq[:, alo - h * 512:ahi - h * 512],
                                bt[:, alo:ahi])
                        nc.scalar.activation(
                            out=sb_PT[:, tj, h * 512:(h + 1) * 512],
                            in_=pq[:],
                            func=mybir.ActivationFunctionType.Exp)

                def phase_AV(tj, po):
                    for ic in range(4):
                        nc.tensor.matmul(
                            po[ic][:],
                            lhsT=sb_vaug[:, tj, :],
                            rhs=sb_PT[:, tj, ic * 512:(ic + 1) * 512],
                            start=(tj == 0), stop=(tj == NT - 1))

                # ---- A block (lead: 12 of 16 tiles) ----
                phase_A(0)
                for c in range(4):
                    pk = psA.tile([128, 512], F32, tag="x")
                    nc.tensor.matmul(
                        pk[0:1, 0:512],
                        lhsT=sb_u8[:],
                        rhs=sb_kaug[0:D, c * 512:(c + 1) * 512],
                        start=True, stop=True)
                    nc.scalar.copy(
                        out=sb_kaug[64:65, c * 512:(c + 1) * 512],
                        in_=pk[0:1, 0:512])
                bts = {}
                for t in range(1, NT):
                    if t >= 9:
                        # issue the xbar ahead of this tile's G write so the
                        # C block's band supply is never queued behind it
                        bts[t - 9] = phase_X(t - 9)
                    phase_A(t)
                bts[7] = phase_X(7)
                psA_ctx.__exit__(None, None, None)

                # ---- C block ----
                ps2_ctx = tc.tile_pool(name="ps2", bufs=4, space="PSUM")
                ps2 = ps2_ctx.__enter__()
                pso_ctx = tc.tile_pool(name="ps_o", bufs=1, space="PSUM")
                pso = pso_ctx.__enter__()
                po = [pso.tile([65, 512], F32, tag=f"po{ic}",
                               name=f"po{ic}")
                      for ic in range(4)]
                # attn@v runs one j-tile behind the logits pipeline so its
                # exp inputs are always ready when the PE reaches it.
                for tj in range(NT):
                    if tj + 8 < NT:
                        bts[tj + 8] = phase_X(tj + 8)
                    phase_C(tj, *bts.pop(tj))
                    if tj >= 1:
                        phase_AV(tj - 1, po)
                phase_AV(NT - 1, po)

                # ---- output fixup ----
                # DVE: po evac + reciprocals; ScalarE: 1/den multiplies
                # (reads pf straight from PSUM); sync: output writes.
                for ic in range(4):
                    nc.vector.tensor_copy(
                        sb_oTD[:, ic * 512:(ic + 1) * 512], po[ic][:])
                for t in range(NT):
                    pf = pso.tile([128, 65], F32, tag=f"po{t % 4}")
                    nc.tensor.transpose(pf[:, 0:65],
                                        sb_oTD[:, t * 128:(t + 1) * 128],
                                        sb_id[0:65, 0:65])
                    rc = fsb.tile([128, 1], F32, tag="rc")
                    nc.vector.reciprocal(rc[:], pf[:, 64:65])
                    ot = fsb.tile([128, D], F32, tag="ot")
                    nc.scalar.mul(ot[:], pf[:, 0:D], rc[:])
                    nc.sync.dma_start(out=d_out[t * 128:(t + 1) * 128, :],
                                      in_=ot[:])
                pso_ctx.__exit__(None, None, None)
                ps2_ctx.__exit__(None, None, None)

    nc.finalize()
    _NC_CACHE["nc"] = nc
    return nc


def _host_prep(query, key, value, u, v, w):
    """Build the 8 per-core input maps from the full inputs."""
    q = np.asarray(query, np.float32)[0]   # [S,H,D]
    k = np.asarray(key, np.float32)[0]
    val = np.asarray(value, np.float32)[0]
    u = np.asarray(u, np.float32)
    v = np.asarray(v, np.float32)
    w = np.asarray(w, np.float32)
    Rr = _basis_feature_matrix()           # [Q, 64]

    ones = np.ones((1, S), np.float32)
    in_maps = []
    for h in range(H):
        qT8 = np.ascontiguousarray(q[:, h, :].T) / np.float32(8.0)  # [64,S]
        qT_aug = np.concatenate([qT8, ones], axis=0)                # [65,S]
        kT = np.ascontiguousarray(k[:, h, :].T)                     # [64,S]
        u8 = (u[h] / np.float32(8.0)).reshape(D, 1)
        vb = val[:, h, :].astype(ml_dtypes.bfloat16)                # [S,64]
        w2r_qr = w[h] @ Rr.T                                        # [64,Q]
        vw8 = (v[h] @ w[h]) / np.float32(8.0)                       # [64]
        w2r_vr = (vw8 @ Rr.T).reshape(1, Q)                         # [1,Q]
        w2r = np.concatenate([w2r_qr, w2r_vr],
                             axis=0).astype(ml_dtypes.bfloat16)
        in_maps.append({
            "qT_bf": np.ascontiguousarray(qT_aug).astype(ml_dtypes.bfloat16),
            "kTb": kT.astype(ml_dtypes.bfloat16),
            "u8b": np.ascontiguousarray(u8).astype(ml_dtypes.bfloat16),
            "vb": np.ascontiguousarray(vb),
            "w2r": np.ascontiguousarray(w2r),
        })
    return in_maps


def kernel(query, key, value, u, v, w, _trace=False):
    nc = _build_nc()
    in_maps = _host_prep(query, key, value, u, v, w)
    res = run_bass_kernel_spmd(nc, in_maps, core_ids=list(range(H)),
                               trace=_trace)
    outs = np.stack([res.results[h]["out"] for h in range(H)])  # [H,S,D]
    full = np.transpose(outs, (1, 0, 2))[None]                  # [1,S,H,D]
    out = np.ascontiguousarray(full.astype(np.float32))
    if _trace:
        return out, res
    return out


if __name__ == "__main__":
    rng = np.random.default_rng(0)
    ins = {
        "query": rng.standard_normal((1, S, H, D), np.float32),
        "key": rng.standard_normal((1, S, H, D), np.float32),
        "value": rng.standard_normal((1, S, H, D), np.float32),
        "u": rng.standard_normal((H, D), np.float32),
        "v": rng.standard_normal((H, D), np.float32),
        "w": rng.standard_normal((H, D, NB), np.float32),
    }
    out = kernel(**ins)
    print("out shape:", out.shape, "finite:", np.isfinite(out).all())
